# revision 14
# baseline (speedup 1.0000x reference)
"""Fused 2-layer GCN forward (nn_Net_SSL_38740605010537) on 8 Trainium2
NeuronCores - scatter-free two-stage aggregation, single launch.

out = log_softmax(A @ relu(A @ (x@W1) + b1) @ W2 + b2),
A = D^-1/2 (Adj+I) D^-1/2.  x is pre-scaled by D^-1/2 on the host so the
matmul emits pre-scaled messages directly.

Per core, per layer: x@W1 on PE -> per-strip AllGather of the message
table -> stage A: windowed dma_gather of per-edge source rows + DVE
segment-reduce per dest-group instance + one contiguous partial-row DMA
write per window (no scatter-add, no RMW hazard, no barriers) ->
stage B: per strip, dests sorted by instance count (host permutation,
undone on the host), one dma_gather per 128-dest group collecting that
group's partial rows plus the dense self-loop row, DVE reduce, fused
epilogue.  Layer-2 classifier + batched log_softmax as before.
"""
import hashlib
import os

import numpy as np

N_NODES, N_EDGES = 100000, 1600000
F_IN, HID, N_CLS = 256, 64, 40
NC, PER_CORE = 8, 12500
NB = 4
SJ, STRIP = 3125, 3200
CHUNK = 4 * STRIP            # 12800
BUCKET_ROWS = 8 * STRIP      # 25600
WCOLS = 48
GCALL = 8
P = 128
NG2 = STRIP // P             # 25 dest groups per strip
F = HID


def _wrap16(flat):
    n = flat.shape[0]
    assert n % 16 == 0
    w = flat.reshape(n // 16, 16).T
    return np.tile(w, (8, 1)).astype(np.int16)


def _pack_layer(src_core, src_b, src_r, dst_core, dst_sd, dst_j, dinv_slot):
    """Pack one layer's aggregation schedule (see module docstring)."""
    E = src_core.shape[0]
    sloc_e = src_core * STRIP + src_r

    cnt = np.zeros((NC, NB, 4, STRIP), np.int64)
    np.add.at(cnt, (dst_core, src_b, dst_sd, dst_j), 1)

    Dlist = {}
    membs = {}
    for b in range(NB):
        for sd in range(4):
            svals, sjls = [], []
            for c in range(NC):
                cc = cnt[c, b, sd]
                nz = np.nonzero(cc)[0]
                o = np.argsort(-cc[nz], kind="stable")
                svals.append(cc[nz][o])
                sjls.append(nz[o])
            gmax = max((len(v) + P - 1) // P for v in svals)
            prof = np.zeros((NC, gmax), np.int64)
            for c in range(NC):
                v = svals[c]
                r = np.arange(gmax) * P
                m = r < len(v)
                prof[c, m] = v[r[m]]
            base = np.maximum(np.floor(prof.mean(axis=0)).astype(np.int64), 1)
            t_vals, t_jls = [], []
            for c in range(NC):
                v, jl = svals[c], sjls[c]
                Dv = base[np.arange(len(v)) // P]
                take = np.minimum(v, Dv)
                left = v - take
                keep = left > 0
                t_vals.append(left[keep])
                t_jls.append(jl[keep])
            tso, tsj = [], []
            for c in range(NC):
                o = np.argsort(-t_vals[c], kind="stable")
                tso.append(t_vals[c][o])
                tsj.append(t_jls[c][o])
            tgmax = max((len(v) + P - 1) // P for v in tso) if any(
                len(v) for v in tso) else 0
            tprof = np.zeros(tgmax, np.int64)
            for g in range(tgmax):
                tprof[g] = max((v[g * P] if g * P < len(v) else 0)
                               for v in tso)
            Dlist[(b, sd)] = list(base[:gmax]) + [int(x) for x in tprof]
            for c in range(NC):
                v, jl = svals[c], sjls[c]
                Dv = base[np.arange(len(v)) // P]
                take_m = np.minimum(v, Dv)
                g_m = np.arange(len(v)) // P
                p_m = np.arange(len(v)) % P
                tv, tj = tso[c], tsj[c]
                g_t = gmax + np.arange(len(tv)) // P
                p_t = np.arange(len(tv)) % P
                membs[(c, b, sd)] = (
                    np.concatenate([jl, tj]),
                    np.concatenate([take_m, tv]),
                    np.concatenate([g_m, g_t]),
                    np.concatenate([p_m, p_t]))

    # ---- strip-pure windows: for sd, for b, first-fit whole groups
    wins = [[] for _ in range(4)]
    colbase = {}
    for sd in range(4):
        for b in range(NB):
            cur, used = [], 0
            for g, D in enumerate(Dlist[(b, sd)]):
                D = int(D)
                assert D <= WCOLS, f"group too wide: {D}"
                if used + D > WCOLS:
                    wins[sd].append(dict(b=b, cols=used, segs=list(cur)))
                    cur, used = [], 0
                colbase[(b, sd, g)] = (sd, len(wins[sd]), used)
                cur.append((g, used, D))
                used += D
            if cur:
                wins[sd].append(dict(b=b, cols=used, segs=list(cur)))
    woff = {}
    gc = 0
    for sd in range(4):
        for w, win in enumerate(wins[sd]):
            woff[(sd, w)] = gc
            gc += win["cols"]
    TOTCOL = gc
    for k, (sd, w, c0) in list(colbase.items()):
        colbase[k] = woff[(sd, w)] + c0

    # ---- partial-row layout per strip: [b0 grps | b1 | b2 | b3 | selves]
    gbase = {}
    rows_sd = []
    self_base = []
    for sd in range(4):
        off = 0
        for b in range(NB):
            gbase[(b, sd)] = off
            off += P * len(Dlist[(b, sd)])
        self_base.append(off)
        rows_sd.append(off + STRIP)
    assert max(rows_sd) < 32768, rows_sd

    # ---- edge -> grid slot assignment
    DUMMY = SJ
    grid = np.full((NC, P, TOTCOL), DUMMY, np.int16)
    key_e = ((dst_core * NB + src_b) * 4 + dst_sd) * STRIP + dst_j
    order_e = np.argsort(key_e, kind="stable")
    ks = key_e[order_e]
    starts = np.r_[0, np.nonzero(np.diff(ks))[0] + 1]
    run_start = np.zeros(len(ks), np.int64)
    run_start[starts] = starts
    run_start = np.maximum.accumulate(run_start)
    q_sorted = np.arange(len(ks)) - run_start

    e_ptr = 0
    eb = np.bincount(key_e // STRIP, minlength=NC * NB * 4)
    for c in range(NC):
        for b in range(NB):
            for sd in range(4):
                n_e = eb[(c * NB + b) * 4 + sd]
                if n_e == 0:
                    continue
                sel = slice(e_ptr, e_ptr + n_e)
                e_ptr += n_e
                jl_m, tk_m, g_m, p_m = membs[(c, b, sd)]
                o = np.lexsort((g_m, jl_m))
                jl_s, tk_s, g_s, p_s = jl_m[o], tk_m[o], g_m[o], p_m[o]
                cum = np.cumsum(tk_s)
                jl_e2 = dst_j[order_e[sel]]
                q_e = q_sorted[sel]
                cc = cnt[c, b, sd]
                cnt_off = np.zeros(STRIP, np.int64)
                cnt_off[1:] = np.cumsum(cc)[:-1]
                e_pos = cnt_off[jl_e2] + q_e
                mi = np.searchsorted(cum, e_pos, side="right")
                colw = e_pos - (cum[mi] - tk_s[mi])
                cb = np.array([colbase[(b, sd, gg)]
                               for gg in range(len(Dlist[(b, sd)]))],
                              np.int64)
                gcol_e = cb[g_s[mi]] + colw
                grid[c, p_s[mi], gcol_e] = sloc_e[order_e[sel]]
    assert e_ptr == E

    gidx = np.empty((NC, P, TOTCOL * 8), np.int16)
    for c in range(NC):
        gidx[c] = _wrap16(grid[c].T.reshape(-1))

    # ---- stage B: per (c, sd) instance rows per dest label, sorted grids
    inst_rows = [[[[] for _ in range(STRIP)] for _ in range(4)]
                 for _ in range(NC)]
    for c in range(NC):
        for sd in range(4):
            for b in range(NB):
                jl_m, _tk, g_m, p_m = membs[(c, b, sd)]
                rows = gbase[(b, sd)] + g_m * P + p_m
                lst = inst_rows[c][sd]
                for j, r in zip(jl_m, rows):
                    lst[j].append(int(r))

    pi = np.empty((NC, 4, STRIP), np.int64)
    D2 = np.empty((NC, 4, NG2), np.int64)
    dv_out = np.zeros((NC, P, 4 * NG2), np.float32)
    for c in range(NC):
        counts = np.array([[len(inst_rows[c][sd][j]) for j in range(STRIP)]
                           for sd in range(4)])
        for sd in range(4):
            order = np.argsort(-counts[sd], kind="stable")
            pi[c, sd] = order
            for gl in range(NG2):
                sel = order[gl * P:(gl + 1) * P]
                D2[c, sd, gl] = counts[sd][sel].max() + 1
                dv_out[c, :, sd * NG2 + gl] = dinv_slot[c, sd][sel]
    D2s = D2.max(axis=0)

    # ---- stage-B windows: pack whole groups, pad window cols to 8 so
    # every gather call is 8-col aligned and 1024-idx shaped (matches the
    # stage-A call structure the hardware accepts).
    WC2 = 24
    wins2 = [[] for _ in range(4)]
    for sd in range(4):
        cur, used = [], 0
        for gl in range(NG2):
            D = int(D2s[sd, gl])
            assert D <= WC2
            if used + D > WC2:
                wins2[sd].append(dict(cols=used,
                                      cols_pad=(used + 7) // 8 * 8,
                                      segs=list(cur)))
                cur, used = [], 0
            cur.append((gl, used, D))
            used += D
        if cur:
            wins2[sd].append(dict(cols=used, cols_pad=(used + 7) // 8 * 8,
                                  segs=list(cur)))
    woff2 = {}
    gc2 = 0
    for sd in range(4):
        for w, win in enumerate(wins2[sd]):
            woff2[(sd, w)] = gc2
            gc2 += win["cols_pad"]
    COLS2 = gc2
    cols2_sd = [sum(w["cols_pad"] for w in wins2[sd]) for sd in range(4)]

    grid2 = np.empty((NC, P, COLS2), np.int64)
    for c in range(NC):
        for sd in range(4):
            sb = self_base[sd]
            dummy = sb + SJ
            order = pi[c, sd]
            for w, win in enumerate(wins2[sd]):
                base = woff2[(sd, w)]
                grid2[c, :, base:base + win["cols_pad"]] = dummy
                for (gl, c0, D) in win["segs"]:
                    block = np.full((P, D), dummy, np.int64)
                    for p in range(P):
                        j = order[gl * P + p]
                        rows = inst_rows[c][sd][j]
                        block[p, 0] = sb + j
                        block[p, 1:1 + len(rows)] = rows
                    grid2[c, :, base + c0:base + c0 + D] = block
    gidx2 = np.empty((NC, P, COLS2 * 8), np.int16)
    for c in range(NC):
        gidx2[c] = _wrap16(grid2[c].T.reshape(-1))

    return dict(Dlist=Dlist, wins=wins, woff=woff, TOTCOL=TOTCOL,
                gbase=gbase, rows_sd=rows_sd, self_base=self_base,
                gidx=gidx, D2s=D2s, cols2_sd=cols2_sd, COLS2=COLS2,
                wins2=wins2, woff2=woff2, WC2=WC2,
                gidx2=gidx2, pi=pi, dv_out=dv_out, grid=grid, grid2=grid2)


def build_schedule(edge_index):
    src = np.asarray(edge_index[0], dtype=np.int64)
    dst = np.asarray(edge_index[1], dtype=np.int64)
    deg = np.bincount(dst, minlength=N_NODES).astype(np.float64) + 1.0
    dinv = (deg ** -0.5).astype(np.float32)

    co = src // PER_CORE
    ii = src % PER_CORE
    b_e = ii // SJ
    r_e = ii % SJ
    cd = dst // PER_CORE
    jj = dst % PER_CORE
    sd_e = jj // SJ
    j_e = jj % SJ

    dv1_slot = np.zeros((NC, 4, STRIP), np.float32)
    for c in range(NC):
        for sd in range(4):
            n0 = c * PER_CORE + sd * SJ
            dv1_slot[c, sd, :SJ] = dinv[n0:n0 + SJ]

    L1 = _pack_layer(co, b_e, r_e, cd, sd_e, j_e, dv1_slot)

    pi1 = L1["pi"]
    pi1_pos = np.empty_like(pi1)
    for c in range(NC):
        for sd in range(4):
            pi1_pos[c, sd, pi1[c, sd]] = np.arange(STRIP)

    r2_e = pi1_pos[co, b_e, r_e]
    j2_e = pi1_pos[cd, sd_e, j_e]
    dv2_slot = np.zeros((NC, 4, STRIP), np.float32)
    for c in range(NC):
        for sd in range(4):
            dv2_slot[c, sd] = dv1_slot[c, sd][pi1[c, sd]]

    L2 = _pack_layer(co, b_e, r2_e, cd, sd_e, j2_e, dv2_slot)

    pi2 = L2["pi"]
    pi2_pos = np.empty_like(pi2)
    for c in range(NC):
        for sd in range(4):
            pi2_pos[c, sd, pi2[c, sd]] = np.arange(STRIP)
    outrow = np.empty((NC, PER_CORE), np.int64)
    for c in range(NC):
        for sd in range(4):
            j = np.arange(SJ)
            outrow[c, sd * SJ:(sd + 1) * SJ] = (
                sd * STRIP + pi2_pos[c, sd, pi1_pos[c, sd, j]])

    return dict(L1=L1, L2=L2, dinv=dinv, outrow=outrow)




class BassRunner:
    """Jit-once PJRT runner for a finalized bass module on 8 cores."""

    def __init__(self, nc, n_cores=8):
        import jax
        from jax.sharding import Mesh, PartitionSpec
        from jax.experimental.shard_map import shard_map
        import concourse.mybir as mybir
        from concourse import bass2jax
        from concourse.bass2jax import _bass_exec_p, partition_id_tensor

        bass2jax.install_neuronx_cc_hook()
        self.jax = jax
        self.nc = nc
        self.n_cores = n_cores
        partition_name = (nc.partition_id_tensor.name
                          if nc.partition_id_tensor else None)
        in_names, out_names, out_avals, zero_outs = [], [], [], []
        for alloc in nc.m.functions[0].allocations:
            if not isinstance(alloc, mybir.MemoryLocationSet):
                continue
            name = alloc.memorylocations[0].name
            if alloc.kind == "ExternalInput":
                if name != partition_name:
                    in_names.append(name)
            elif alloc.kind == "ExternalOutput":
                shape = tuple(alloc.tensor_shape)
                dtype = mybir.dt.np(alloc.dtype)
                out_avals.append(jax.core.ShapedArray(shape, dtype))
                out_names.append(name)
                zero_outs.append(np.zeros(shape, dtype))
        self.in_names = list(in_names)
        self.out_names = out_names
        self.out_avals = out_avals
        self.zero_outs = zero_outs
        n_params = len(self.in_names)
        n_outs = len(out_names)
        all_in_names = self.in_names + out_names
        if partition_name is not None:
            all_in_names.append(partition_name)

        def _body(*args):
            operands = list(args)
            if partition_name is not None:
                operands.append(partition_id_tensor())
            outs = _bass_exec_p.bind(
                *operands,
                out_avals=tuple(out_avals),
                in_names=tuple(all_in_names),
                out_names=tuple(out_names),
                lowering_input_output_aliases=(),
                sim_require_finite=True,
                sim_require_nnan=True,
                nc=nc,
            )
            return tuple(outs)

        devices = jax.devices()[:n_cores]
        self.mesh = Mesh(np.asarray(devices), ("core",))
        in_specs = (PartitionSpec("core"),) * (n_params + n_outs)
        out_specs = (PartitionSpec("core"),) * n_outs
        self.donate = (() if os.environ.get("BASS_NO_DONATE")
                       else tuple(range(n_params, n_params + n_outs)))
        self.fn = jax.jit(
            shard_map(_body, mesh=self.mesh, in_specs=in_specs,
                      out_specs=out_specs, check_rep=False),
            donate_argnums=self.donate, keep_unused=True,
        )
        self.sharding = jax.sharding.NamedSharding(self.mesh,
                                                   PartitionSpec("core"))

    def put_inputs(self, in_maps):
        concat = []
        for name in self.in_names:
            arr = np.concatenate([np.asarray(m[name]) for m in in_maps], axis=0)
            concat.append(self.jax.device_put(arr, self.sharding))
        return concat

    def _zeros(self):
        return [self.jax.device_put(
                    np.zeros((self.n_cores * z.shape[0], *z.shape[1:]), z.dtype),
                    self.sharding)
                for z in self.zero_outs]

    def run(self, dev_inputs):
        outs = self.fn(*dev_inputs, *self._zeros())
        self.jax.block_until_ready(outs)
        return outs

    def time_runs(self, dev_inputs, n_rep=6):
        import time
        ts = []
        for _ in range(n_rep):
            zeros = self._zeros()
            self.jax.block_until_ready(zeros)
            t0 = time.monotonic()
            outs = self.fn(*dev_inputs, *zeros)
            self.jax.block_until_ready(outs)
            ts.append(time.monotonic() - t0)
        return min(ts), ts

    def results(self, outs):
        res = []
        for c in range(self.n_cores):
            d = {}
            for i, name in enumerate(self.out_names):
                d[name] = np.asarray(outs[i]).reshape(
                    self.n_cores, *self.out_avals[i].shape)[c]
            res.append(d)
        return res




_runners = {}
_prep_cache = {}


def _build(meta):
    import concourse.bacc as bacc
    import concourse.tile as tile
    from concourse import mybir
    from concourse.masks import make_identity

    L1, L2 = meta["L1"], meta["L2"]
    nc = bacc.Bacc(None, target_bir_lowering=False, num_devices=NC,
                   num_swdge_queues=4, dynamic_dma_scratch_size=2 ** 15)
    xT = nc.dram_tensor("xT", [F_IN, CHUNK], mybir.dt.float32,
                        kind="ExternalInput")
    w1 = nc.dram_tensor("w1", [F_IN, HID], mybir.dt.float32,
                        kind="ExternalInput")
    w2 = nc.dram_tensor("w2", [HID, N_CLS], mybir.dt.float32,
                        kind="ExternalInput")
    b1d = nc.dram_tensor("b1d", [P, HID], mybir.dt.float32,
                         kind="ExternalInput")
    b2d = nc.dram_tensor("b2d", [P, N_CLS], mybir.dt.float32,
                         kind="ExternalInput")
    dv1d = nc.dram_tensor("dv1d", [P, 4 * NG2], mybir.dt.float32,
                          kind="ExternalInput")
    dv2d = nc.dram_tensor("dv2d", [P, 4 * NG2], mybir.dt.float32,
                          kind="ExternalInput")
    gA1d = nc.dram_tensor("gA1d", [P, L1["TOTCOL"] * 8], mybir.dt.int16,
                          kind="ExternalInput")
    gB1d = nc.dram_tensor("gB1d", [P, L1["COLS2"] * 8], mybir.dt.int16,
                          kind="ExternalInput")
    gA2d = nc.dram_tensor("gA2d", [P, L2["TOTCOL"] * 8], mybir.dt.int16,
                          kind="ExternalInput")
    gB2d = nc.dram_tensor("gB2d", [P, L2["COLS2"] * 8], mybir.dt.int16,
                          kind="ExternalInput")
    outd = nc.dram_tensor("outd", [CHUNK, N_CLS], mybir.dt.float32,
                          kind="ExternalOutput")

    agin1 = [nc.dram_tensor(f"agin1_{k}", [STRIP, F], mybir.dt.float32,
                            kind="Internal") for k in range(4)]
    agin2 = [nc.dram_tensor(f"agin2_{k}", [STRIP, F], mybir.dt.float32,
                            kind="Internal") for k in range(4)]
    table1 = [nc.dram_tensor(f"table1_{k}", [BUCKET_ROWS, F],
                             mybir.dt.float32, kind="Internal",
                             addr_space="Shared") for k in range(4)]
    table2 = [nc.dram_tensor(f"table2_{k}", [BUCKET_ROWS, F],
                             mybir.dt.float32, kind="Internal",
                             addr_space="Shared") for k in range(4)]
    pbuf1 = [nc.dram_tensor(f"pbuf1_{k}", [L1["rows_sd"][k], F],
                            mybir.dt.float32, kind="Internal")
             for k in range(4)]
    pbuf2 = [nc.dram_tensor(f"pbuf2_{k}", [L2["rows_sd"][k], F],
                            mybir.dt.float32, kind="Internal")
             for k in range(4)]

    with tile.TileContext(nc) as tc:
        with tc.tile_pool(name="c0", bufs=1) as cp, \
             tc.tile_pool(name="xs", bufs=2) as xsp, \
             tc.tile_pool(name="mm", bufs=4) as mmp, \
             tc.tile_pool(name="gw", bufs=4) as gwp, \
             tc.tile_pool(name="sb", bufs=4) as sbp, \
             tc.tile_pool(name="gi", bufs=4) as gip, \
             tc.tile_pool(name="g2", bufs=4) as g2p, \
             tc.tile_pool(name="i2", bufs=2) as i2p, \
             tc.tile_pool(name="ep", bufs=4) as epp, \
             tc.tile_pool(name="p1", bufs=4, space="PSUM") as pp1, \
             tc.tile_pool(name="p2", bufs=2, space="PSUM") as pp2:

            # ---- constants
            w1a = cp.tile([P, HID], mybir.dt.float32, tag="w1a")
            w1b = cp.tile([P, HID], mybir.dt.float32, tag="w1b")
            nc.sync.dma_start(out=w1a[:], in_=w1[0:P, :])
            nc.sync.dma_start(out=w1b[:], in_=w1[P:2 * P, :])
            w2t = cp.tile([HID, N_CLS], mybir.dt.float32, tag="w2t")
            nc.sync.dma_start(out=w2t[:], in_=w2[:])
            b1t = cp.tile([P, HID], mybir.dt.float32, tag="b1t")
            nc.sync.dma_start(out=b1t[:], in_=b1d[:])
            b2t = cp.tile([P, N_CLS], mybir.dt.float32, tag="b2t")
            nc.sync.dma_start(out=b2t[:], in_=b2d[:])
            dv1 = cp.tile([P, 4 * NG2], mybir.dt.float32, tag="dv1")
            nc.sync.dma_start(out=dv1[:], in_=dv1d[:])
            dv2 = cp.tile([P, 4 * NG2], mybir.dt.float32, tag="dv2")
            nc.sync.dma_start(out=dv2[:], in_=dv2d[:])
            ident = cp.tile([P, P], mybir.dt.float32, tag="id")
            make_identity(nc, ident[:])

            # ---- layer-1 matmul: hp strips -> agin1 + pbuf1 self block
            sb1 = L1["self_base"]
            for ch in range(10):
                xa = xsp.tile([P, 1280], mybir.dt.float32, tag="xa")
                xb = xsp.tile([P, 1280], mybir.dt.float32, tag="xb")
                nc.sync.dma_start(out=xa[:],
                                  in_=xT[0:P, ch * 1280:(ch + 1) * 1280])
                nc.sync.dma_start(out=xb[:],
                                  in_=xT[P:2 * P, ch * 1280:(ch + 1) * 1280])
                for gl in range(10):
                    g = ch * 10 + gl
                    pst = pp1.tile([P, HID], mybir.dt.float32, tag="mmps")
                    nc.tensor.matmul(out=pst[:],
                                     lhsT=xa[:, gl * P:(gl + 1) * P],
                                     rhs=w1a[:], start=True, stop=False)
                    nc.tensor.matmul(out=pst[:],
                                     lhsT=xb[:, gl * P:(gl + 1) * P],
                                     rhs=w1b[:], start=False, stop=True)
                    ot = mmp.tile([P, HID], mybir.dt.float32, tag="ot")
                    nc.vector.tensor_copy(out=ot[:], in_=pst[:])
                    sd, gl2 = g // 25, g % 25
                    nc.sync.dma_start(
                        out=agin1[sd][gl2 * P:(gl2 + 1) * P, :], in_=ot[:])
                    nc.sync.dma_start(
                        out=pbuf1[sd][sb1[sd] + gl2 * P:
                                      sb1[sd] + (gl2 + 1) * P, :], in_=ot[:])
                    if g % 25 == 24:
                        nc.gpsimd.collective_compute(
                            "AllGather", mybir.AluOpType.bypass,
                            replica_groups=[list(range(NC))],
                            ins=[agin1[sd][:].opt()],
                            outs=[table1[sd][:].opt()])

            # ---- stage A for one (layer, strip): windows
            def stage_a(L, tables, pbufs, sd):
                for w, win in enumerate(L["wins"][sd]):
                    b = win["b"]
                    cols = win["cols"]
                    wc0 = L["woff"][(sd, w)]
                    gsrc = gA1d if L is L1 else gA2d
                    git = gip.tile([P, WCOLS * 8], mybir.dt.int16, tag="git")
                    nc.scalar.dma_start(
                        out=git[:, :cols * 8],
                        in_=gsrc[:, wc0 * 8:(wc0 + cols) * 8])
                    gwt = gwp.tile([P, WCOLS, F], mybir.dt.float32, tag="gw")
                    for cc in range(0, cols, GCALL):
                        k8 = min(GCALL, cols - cc)
                        nc.gpsimd.dma_gather(
                            out_ap=gwt[:, cc:cc + k8, :],
                            in_ap=tables[b][:],
                            idxs_ap=git[:, cc * 8:(cc + k8) * 8],
                            num_idxs=k8 * P, num_idxs_reg=k8 * P,
                            elem_size=F, queue_num=0)
                    nsegs = len(win["segs"])
                    sbt = sbp.tile([P, WCOLS, F], mybir.dt.float32, tag="sb")
                    for r, (g, c0, d) in enumerate(win["segs"]):
                        nc.vector.tensor_reduce(
                            out=sbt[:, r, :],
                            in_=gwt[:, c0:c0 + d, :].rearrange(
                                "p g f -> p f g"),
                            axis=mybir.AxisListType.X,
                            op=mybir.AluOpType.add)
                    g0 = win["segs"][0][0]
                    base = L["gbase"][(b, sd)] + g0 * P
                    nc.sync.dma_start(
                        out=pbufs[sd][base:base + nsegs * P, :].rearrange(
                            "(r p) f -> p r f", p=P),
                        in_=sbt[:, :nsegs, :])

            # ---- stage B layer 1: totals -> relu epilogue -> agin2/pbuf2
            sb2 = L2["self_base"]

            def load_git2(L, gsrc, sd):
                c2 = L["cols2_sd"][sd]
                off = sum(L["cols2_sd"][:sd])
                git2 = i2p.tile([P, max(L["cols2_sd"]) * 8], mybir.dt.int16,
                                tag="git2" + ("a" if L is L1 else "b"))
                nc.scalar.dma_start(out=git2[:, :c2 * 8],
                                    in_=gsrc[:, off * 8:(off + c2) * 8])
                return git2

            WC2 = L1["WC2"]

            def win2_gather(L, pbufs, git2, sd, w, win):
                """Gather one stage-B window; returns the window tile."""
                off = sum(L["cols2_sd"][:sd])
                wc0 = L["woff2"][(sd, w)] - off  # strip-local col base
                cols = win["cols_pad"]
                g2t = g2p.tile([P, WC2, F], mybir.dt.float32, tag="g2t")
                for cc in range(0, cols, GCALL):
                    k8 = min(GCALL, cols - cc)
                    nc.gpsimd.dma_gather(
                        out_ap=g2t[:, cc:cc + k8, :], in_ap=pbufs[sd][:],
                        idxs_ap=git2[:, (wc0 + cc) * 8:(wc0 + cc + k8) * 8],
                        num_idxs=k8 * P, num_idxs_reg=k8 * P,
                        elem_size=F, queue_num=0)
                return g2t

            def seg_total(g2t, c0, D):
                t1 = epp.tile([P, F], mybir.dt.float32, tag="t1")
                nc.vector.tensor_reduce(
                    out=t1[:],
                    in_=g2t[:, c0:c0 + D, :].rearrange("p g f -> p f g"),
                    axis=mybir.AxisListType.X,
                    op=mybir.AluOpType.add)
                return t1

            def stage_b1(sd):
                git2 = load_git2(L1, gB1d, sd)
                for w, win in enumerate(L1["wins2"][sd]):
                    g2t = win2_gather(L1, pbuf1, git2, sd, w, win)
                    for (gl, c0, D) in win["segs"]:
                        g = sd * NG2 + gl
                        dcol = dv1[:, g:g + 1]
                        t1 = seg_total(g2t, c0, D)
                        nc.vector.tensor_tensor(
                            out=t1[:], in0=t1[:],
                            in1=dcol.to_broadcast([P, F]),
                            op=mybir.AluOpType.mult)
                        nc.vector.tensor_tensor(
                            out=t1[:], in0=t1[:], in1=b1t[:],
                            op=mybir.AluOpType.add)
                        t2 = epp.tile([P, F], mybir.dt.float32, tag="t2")
                        nc.scalar.activation(
                            out=t2[:], in_=t1[:],
                            func=mybir.ActivationFunctionType.Relu)
                        nc.vector.tensor_tensor(
                            out=t2[:], in0=t2[:],
                            in1=dcol.to_broadcast([P, F]),
                            op=mybir.AluOpType.mult)
                        nc.sync.dma_start(
                            out=agin2[sd][gl * P:(gl + 1) * P, :], in_=t2[:])
                        nc.sync.dma_start(
                            out=pbuf2[sd][sb2[sd] + gl * P:
                                          sb2[sd] + (gl + 1) * P, :],
                            in_=t2[:])
                nc.gpsimd.collective_compute(
                    "AllGather", mybir.AluOpType.bypass,
                    replica_groups=[list(range(NC))],
                    ins=[agin2[sd][:].opt()], outs=[table2[sd][:].opt()])

            # ---- stage B layer 2: classifier + batched log_softmax
            lgbuf = cp.tile([P, 4 * NG2 * N_CLS], mybir.dt.float32, tag="lgb")
            smT = cp.tile([P, 4 * NG2], mybir.dt.float32, tag="smT")
            lnT = cp.tile([P, 4 * NG2], mybir.dt.float32, tag="lnT")

            def stage_b2(sd):
                git2 = load_git2(L2, gB2d, sd)
                for w, win in enumerate(L2["wins2"][sd]):
                    g2t = win2_gather(L2, pbuf2, git2, sd, w, win)
                    for (gl, c0, D) in win["segs"]:
                        g = sd * NG2 + gl
                        dcol = dv2[:, g:g + 1]
                        t1 = seg_total(g2t, c0, D)
                        pt = pp2.tile([HID, P], mybir.dt.float32, tag="pt")
                        nc.tensor.transpose(out=pt[:], in_=t1[:],
                                            identity=ident[:])
                        zt2 = epp.tile([HID, P], mybir.dt.float32, tag="zt2")
                        nc.vector.tensor_copy(out=zt2[:], in_=pt[:])
                        p2t = pp2.tile([P, N_CLS], mybir.dt.float32, tag="p2")
                        nc.tensor.matmul(out=p2t[:], lhsT=zt2[:], rhs=w2t[:],
                                         start=True, stop=True)
                        lg = lgbuf[:, g * N_CLS:(g + 1) * N_CLS]
                        nc.vector.tensor_tensor(
                            out=lg, in0=p2t[:],
                            in1=dcol.to_broadcast([P, N_CLS]),
                            op=mybir.AluOpType.mult)
                        nc.vector.tensor_tensor(out=lg, in0=lg, in1=b2t[:],
                                                op=mybir.AluOpType.add)
                        mx = epp.tile([P, 1], mybir.dt.float32, tag="mx")
                        nc.vector.tensor_reduce(out=mx[:], in_=lg,
                                                axis=mybir.AxisListType.X,
                                                op=mybir.AluOpType.max)
                        nc.vector.tensor_tensor(
                            out=lg, in0=lg,
                            in1=mx[:].to_broadcast([P, N_CLS]),
                            op=mybir.AluOpType.subtract)
                        ex = epp.tile([P, N_CLS], mybir.dt.float32, tag="ex")
                        nc.scalar.activation(
                            out=ex[:], in_=lg,
                            func=mybir.ActivationFunctionType.Exp)
                        nc.vector.tensor_reduce(out=smT[:, g:g + 1],
                                                in_=ex[:],
                                                axis=mybir.AxisListType.X,
                                                op=mybir.AluOpType.add)
                nc.scalar.activation(
                    out=lnT[:, sd * NG2:(sd + 1) * NG2],
                    in_=smT[:, sd * NG2:(sd + 1) * NG2],
                    func=mybir.ActivationFunctionType.Ln)
                for gl in range(NG2):
                    g = sd * NG2 + gl
                    og = epp.tile([P, N_CLS], mybir.dt.float32, tag="og")
                    nc.vector.tensor_tensor(
                        out=og[:], in0=lgbuf[:, g * N_CLS:(g + 1) * N_CLS],
                        in1=lnT[:, g:g + 1].to_broadcast([P, N_CLS]),
                        op=mybir.AluOpType.subtract)
                    nc.sync.dma_start(
                        out=outd[(sd * STRIP + gl * P):
                                 (sd * STRIP + (gl + 1) * P), :],
                        in_=og[:])

            # ---- emission: stage B pipelined one strip behind stage A
            stage_a(L1, table1, pbuf1, 0)
            stage_a(L1, table1, pbuf1, 1)
            stage_b1(0)
            stage_a(L1, table1, pbuf1, 2)
            stage_b1(1)
            stage_a(L1, table1, pbuf1, 3)
            stage_b1(2)
            stage_b1(3)
            stage_a(L2, table2, pbuf2, 0)
            stage_a(L2, table2, pbuf2, 1)
            stage_b2(0)
            stage_a(L2, table2, pbuf2, 2)
            stage_b2(1)
            stage_a(L2, table2, pbuf2, 3)
            stage_b2(2)
            stage_b2(3)
    nc.finalize()
    return nc


def _prep(edge_index):
    key = hashlib.sha1(np.ascontiguousarray(edge_index).tobytes()).hexdigest()
    if key not in _prep_cache:
        _prep_cache[key] = build_schedule(edge_index)
    return _prep_cache[key]


def _meta_key(meta):
    h = hashlib.sha1()
    for L in (meta["L1"], meta["L2"]):
        h.update(repr([L["TOTCOL"], L["COLS2"], L["cols2_sd"],
                       L["rows_sd"], sorted(L["gbase"].items()),
                       L["D2s"].tolist(),
                       [(sd, w["b"], w["cols"], tuple(map(tuple, w["segs"])))
                        for sd in range(4) for w in L["wins"][sd]],
                       [(sd, w["cols"], w["cols_pad"],
                         tuple(map(tuple, w["segs"])))
                        for sd in range(4) for w in L["wins2"][sd]]]).encode())
    return h.hexdigest()


def get_runner(meta):
    key = _meta_key(meta)
    if key not in _runners:
        _runners[key] = BassRunner(_build(meta), NC)
    return _runners[key]


def make_in_maps(x, W1, b1, W2, b2, meta):
    x = np.asarray(x, np.float32)
    dinv = meta["dinv"]
    L1, L2 = meta["L1"], meta["L2"]
    b1r = np.tile(np.asarray(b1, np.float32)[None, :], (P, 1))
    b2r = np.tile(np.asarray(b2, np.float32)[None, :], (P, 1))
    xs_all = (x * dinv[:, None]).astype(np.float32)
    in_maps = []
    for c in range(NC):
        xs = np.zeros((F_IN, CHUNK), np.float32)
        for sd in range(4):
            n0 = c * PER_CORE + sd * SJ
            xs[:, sd * STRIP:sd * STRIP + SJ] = xs_all[n0:n0 + SJ].T
        in_maps.append({
            "xT": xs, "w1": np.asarray(W1, np.float32),
            "w2": np.asarray(W2, np.float32), "b1d": b1r, "b2d": b2r,
            "dv1d": L1["dv_out"][c], "dv2d": L2["dv_out"][c],
            "gA1d": L1["gidx"][c], "gB1d": L1["gidx2"][c],
            "gA2d": L2["gidx"][c], "gB2d": L2["gidx2"][c]})
    return in_maps


def assemble_out(res, meta):
    outrow = meta["outrow"]
    out = np.empty((N_NODES, N_CLS), np.float32)
    for c in range(NC):
        out[c * PER_CORE:(c + 1) * PER_CORE] = res[c]["outd"][outrow[c]]
    return out


def kernel(x, edge_index, pos_edge_index, neg_edge_index, masked_nodes,
           W1, b1, W2, b2):
    meta = _prep(np.asarray(edge_index))
    runner = get_runner(meta)
    in_maps = make_in_maps(x, W1, b1, W2, b2, meta)
    res = runner.results(runner.run(runner.put_inputs(in_maps)))
    return assemble_out(res, meta)


# revision 15
# speedup vs baseline: 2.3214x; 2.3214x over previous
"""Fused 2-layer GCN forward (nn_Net_SSL_38740605010537) on 8 Trainium2
NeuronCores - scatter-free two-stage aggregation, single launch.

out = log_softmax(A @ relu(A @ (x@W1) + b1) @ W2 + b2),
A = D^-1/2 (Adj+I) D^-1/2.  x is pre-scaled by D^-1/2 on the host so the
matmul emits pre-scaled messages directly.

Per core, per layer: x@W1 on PE -> per-strip AllGather of the message
table -> stage A: windowed dma_gather of per-edge source rows + DVE
segment-reduce per dest-group instance + one contiguous partial-row DMA
write per window (no scatter-add, no RMW hazard, no barriers) ->
stage B: per strip, dests sorted by instance count (host permutation,
undone on the host), one dma_gather per 128-dest group collecting that
group's partial rows plus the dense self-loop row, DVE reduce, fused
epilogue.  Layer-2 classifier + batched log_softmax as before.
"""
import hashlib
import os

import numpy as np

N_NODES, N_EDGES = 100000, 1600000
F_IN, HID, N_CLS = 256, 64, 40
NC, PER_CORE = 8, 12500
NB = 4
SJ, STRIP = 3125, 3200
CHUNK = 4 * STRIP            # 12800
BUCKET_ROWS = 8 * STRIP      # 25600
WCOLS = 48
GCALL = 8
P = 128
NG2 = STRIP // P             # 25 dest groups per strip
F = HID


def _wrap16(flat):
    n = flat.shape[0]
    assert n % 16 == 0
    w = flat.reshape(n // 16, 16).T
    return np.tile(w, (8, 1)).astype(np.int16)


def _pack_layer(src_core, src_b, src_r, dst_core, dst_sd, dst_j, dinv_slot):
    """Pack one layer's aggregation schedule (see module docstring)."""
    E = src_core.shape[0]
    sloc_e = src_core * STRIP + src_r

    cnt = np.zeros((NC, NB, 4, STRIP), np.int64)
    np.add.at(cnt, (dst_core, src_b, dst_sd, dst_j), 1)

    Dlist = {}
    membs = {}
    for b in range(NB):
        for sd in range(4):
            svals, sjls = [], []
            for c in range(NC):
                cc = cnt[c, b, sd]
                nz = np.nonzero(cc)[0]
                o = np.argsort(-cc[nz], kind="stable")
                svals.append(cc[nz][o])
                sjls.append(nz[o])
            gmax = max((len(v) + P - 1) // P for v in svals)
            prof = np.zeros((NC, gmax), np.int64)
            for c in range(NC):
                v = svals[c]
                r = np.arange(gmax) * P
                m = r < len(v)
                prof[c, m] = v[r[m]]
            base = np.maximum(np.floor(prof.mean(axis=0)).astype(np.int64), 1)
            t_vals, t_jls = [], []
            for c in range(NC):
                v, jl = svals[c], sjls[c]
                Dv = base[np.arange(len(v)) // P]
                take = np.minimum(v, Dv)
                left = v - take
                keep = left > 0
                t_vals.append(left[keep])
                t_jls.append(jl[keep])
            tso, tsj = [], []
            for c in range(NC):
                o = np.argsort(-t_vals[c], kind="stable")
                tso.append(t_vals[c][o])
                tsj.append(t_jls[c][o])
            tgmax = max((len(v) + P - 1) // P for v in tso) if any(
                len(v) for v in tso) else 0
            tprof = np.zeros(tgmax, np.int64)
            for g in range(tgmax):
                tprof[g] = max((v[g * P] if g * P < len(v) else 0)
                               for v in tso)
            Dlist[(b, sd)] = list(base[:gmax]) + [int(x) for x in tprof]
            for c in range(NC):
                v, jl = svals[c], sjls[c]
                Dv = base[np.arange(len(v)) // P]
                take_m = np.minimum(v, Dv)
                g_m = np.arange(len(v)) // P
                p_m = np.arange(len(v)) % P
                tv, tj = tso[c], tsj[c]
                g_t = gmax + np.arange(len(tv)) // P
                p_t = np.arange(len(tv)) % P
                membs[(c, b, sd)] = (
                    np.concatenate([jl, tj]),
                    np.concatenate([take_m, tv]),
                    np.concatenate([g_m, g_t]),
                    np.concatenate([p_m, p_t]))

    # ---- strip-pure windows: for sd, for b, first-fit whole groups
    wins = [[] for _ in range(4)]
    colbase = {}
    for sd in range(4):
        for b in range(NB):
            cur, used = [], 0
            for g, D in enumerate(Dlist[(b, sd)]):
                D = int(D)
                assert D <= WCOLS, f"group too wide: {D}"
                if used + D > WCOLS:
                    wins[sd].append(dict(b=b, cols=used, segs=list(cur)))
                    cur, used = [], 0
                colbase[(b, sd, g)] = (sd, len(wins[sd]), used)
                cur.append((g, used, D))
                used += D
            if cur:
                wins[sd].append(dict(b=b, cols=used, segs=list(cur)))
    woff = {}
    gc = 0
    for sd in range(4):
        for w, win in enumerate(wins[sd]):
            woff[(sd, w)] = gc
            gc += win["cols"]
    TOTCOL = gc
    for k, (sd, w, c0) in list(colbase.items()):
        colbase[k] = woff[(sd, w)] + c0

    # ---- partial-row layout per strip: [b0 grps | b1 | b2 | b3 | selves]
    gbase = {}
    rows_sd = []
    self_base = []
    for sd in range(4):
        off = 0
        for b in range(NB):
            gbase[(b, sd)] = off
            off += P * len(Dlist[(b, sd)])
        self_base.append(off)
        rows_sd.append(off + STRIP)
    assert max(rows_sd) < 32768, rows_sd

    # ---- edge -> grid slot assignment
    DUMMY = SJ
    grid = np.full((NC, P, TOTCOL), DUMMY, np.int16)
    key_e = ((dst_core * NB + src_b) * 4 + dst_sd) * STRIP + dst_j
    order_e = np.argsort(key_e, kind="stable")
    ks = key_e[order_e]
    starts = np.r_[0, np.nonzero(np.diff(ks))[0] + 1]
    run_start = np.zeros(len(ks), np.int64)
    run_start[starts] = starts
    run_start = np.maximum.accumulate(run_start)
    q_sorted = np.arange(len(ks)) - run_start

    e_ptr = 0
    eb = np.bincount(key_e // STRIP, minlength=NC * NB * 4)
    for c in range(NC):
        for b in range(NB):
            for sd in range(4):
                n_e = eb[(c * NB + b) * 4 + sd]
                if n_e == 0:
                    continue
                sel = slice(e_ptr, e_ptr + n_e)
                e_ptr += n_e
                jl_m, tk_m, g_m, p_m = membs[(c, b, sd)]
                o = np.lexsort((g_m, jl_m))
                jl_s, tk_s, g_s, p_s = jl_m[o], tk_m[o], g_m[o], p_m[o]
                cum = np.cumsum(tk_s)
                jl_e2 = dst_j[order_e[sel]]
                q_e = q_sorted[sel]
                cc = cnt[c, b, sd]
                cnt_off = np.zeros(STRIP, np.int64)
                cnt_off[1:] = np.cumsum(cc)[:-1]
                e_pos = cnt_off[jl_e2] + q_e
                mi = np.searchsorted(cum, e_pos, side="right")
                colw = e_pos - (cum[mi] - tk_s[mi])
                cb = np.array([colbase[(b, sd, gg)]
                               for gg in range(len(Dlist[(b, sd)]))],
                              np.int64)
                gcol_e = cb[g_s[mi]] + colw
                grid[c, p_s[mi], gcol_e] = sloc_e[order_e[sel]]
    assert e_ptr == E

    gidx = np.empty((NC, P, TOTCOL * 8), np.int16)
    for c in range(NC):
        gidx[c] = _wrap16(grid[c].T.reshape(-1))

    # ---- stage B: per (c, sd) instance rows per dest label, sorted grids
    inst_rows = [[[[] for _ in range(STRIP)] for _ in range(4)]
                 for _ in range(NC)]
    for c in range(NC):
        for sd in range(4):
            for b in range(NB):
                jl_m, _tk, g_m, p_m = membs[(c, b, sd)]
                rows = gbase[(b, sd)] + g_m * P + p_m
                lst = inst_rows[c][sd]
                for j, r in zip(jl_m, rows):
                    lst[j].append(int(r))

    pi = np.empty((NC, 4, STRIP), np.int64)
    D2 = np.empty((NC, 4, NG2), np.int64)
    dv_out = np.zeros((NC, P, 4 * NG2), np.float32)
    for c in range(NC):
        counts = np.array([[len(inst_rows[c][sd][j]) for j in range(STRIP)]
                           for sd in range(4)])
        for sd in range(4):
            order = np.argsort(-counts[sd], kind="stable")
            pi[c, sd] = order
            for gl in range(NG2):
                sel = order[gl * P:(gl + 1) * P]
                D2[c, sd, gl] = counts[sd][sel].max() + 1
                dv_out[c, :, sd * NG2 + gl] = dinv_slot[c, sd][sel]
    D2s = D2.max(axis=0)

    # ---- stage-B windows: pack whole groups, pad window cols to 8 so
    # every gather call is 8-col aligned and 1024-idx shaped (matches the
    # stage-A call structure the hardware accepts).
    WC2 = 24
    wins2 = [[] for _ in range(4)]
    for sd in range(4):
        cur, used = [], 0
        for gl in range(NG2):
            D = int(D2s[sd, gl])
            assert D <= WC2
            if used + D > WC2:
                wins2[sd].append(dict(cols=used,
                                      cols_pad=(used + 7) // 8 * 8,
                                      segs=list(cur)))
                cur, used = [], 0
            cur.append((gl, used, D))
            used += D
        if cur:
            wins2[sd].append(dict(cols=used, cols_pad=(used + 7) // 8 * 8,
                                  segs=list(cur)))
    woff2 = {}
    gc2 = 0
    for sd in range(4):
        for w, win in enumerate(wins2[sd]):
            woff2[(sd, w)] = gc2
            gc2 += win["cols_pad"]
    COLS2 = gc2
    cols2_sd = [sum(w["cols_pad"] for w in wins2[sd]) for sd in range(4)]

    grid2 = np.empty((NC, P, COLS2), np.int64)
    for c in range(NC):
        for sd in range(4):
            sb = self_base[sd]
            dummy = sb + SJ
            order = pi[c, sd]
            for w, win in enumerate(wins2[sd]):
                base = woff2[(sd, w)]
                grid2[c, :, base:base + win["cols_pad"]] = dummy
                for (gl, c0, D) in win["segs"]:
                    block = np.full((P, D), dummy, np.int64)
                    for p in range(P):
                        j = order[gl * P + p]
                        rows = inst_rows[c][sd][j]
                        block[p, 0] = sb + j
                        block[p, 1:1 + len(rows)] = rows
                    grid2[c, :, base + c0:base + c0 + D] = block
    gidx2 = np.empty((NC, P, COLS2 * 8), np.int16)
    for c in range(NC):
        gidx2[c] = _wrap16(grid2[c].T.reshape(-1))

    return dict(Dlist=Dlist, wins=wins, woff=woff, TOTCOL=TOTCOL,
                gbase=gbase, rows_sd=rows_sd, self_base=self_base,
                gidx=gidx, D2s=D2s, cols2_sd=cols2_sd, COLS2=COLS2,
                wins2=wins2, woff2=woff2, WC2=WC2,
                gidx2=gidx2, pi=pi, dv_out=dv_out, grid=grid, grid2=grid2)


def build_schedule(edge_index):
    src = np.asarray(edge_index[0], dtype=np.int64)
    dst = np.asarray(edge_index[1], dtype=np.int64)
    deg = np.bincount(dst, minlength=N_NODES).astype(np.float64) + 1.0
    dinv = (deg ** -0.5).astype(np.float32)

    co = src // PER_CORE
    ii = src % PER_CORE
    b_e = ii // SJ
    r_e = ii % SJ
    cd = dst // PER_CORE
    jj = dst % PER_CORE
    sd_e = jj // SJ
    j_e = jj % SJ

    dv1_slot = np.zeros((NC, 4, STRIP), np.float32)
    for c in range(NC):
        for sd in range(4):
            n0 = c * PER_CORE + sd * SJ
            dv1_slot[c, sd, :SJ] = dinv[n0:n0 + SJ]

    L1 = _pack_layer(co, b_e, r_e, cd, sd_e, j_e, dv1_slot)

    pi1 = L1["pi"]
    pi1_pos = np.empty_like(pi1)
    for c in range(NC):
        for sd in range(4):
            pi1_pos[c, sd, pi1[c, sd]] = np.arange(STRIP)

    r2_e = pi1_pos[co, b_e, r_e]
    j2_e = pi1_pos[cd, sd_e, j_e]
    dv2_slot = np.zeros((NC, 4, STRIP), np.float32)
    for c in range(NC):
        for sd in range(4):
            dv2_slot[c, sd] = dv1_slot[c, sd][pi1[c, sd]]

    L2 = _pack_layer(co, b_e, r2_e, cd, sd_e, j2_e, dv2_slot)

    pi2 = L2["pi"]
    pi2_pos = np.empty_like(pi2)
    for c in range(NC):
        for sd in range(4):
            pi2_pos[c, sd, pi2[c, sd]] = np.arange(STRIP)
    outrow = np.empty((NC, PER_CORE), np.int64)
    for c in range(NC):
        for sd in range(4):
            j = np.arange(SJ)
            outrow[c, sd * SJ:(sd + 1) * SJ] = (
                sd * STRIP + pi2_pos[c, sd, pi1_pos[c, sd, j]])

    return dict(L1=L1, L2=L2, dinv=dinv, outrow=outrow)




class BassRunner:
    """Jit-once PJRT runner for a finalized bass module on 8 cores."""

    def __init__(self, nc, n_cores=8):
        import jax
        from jax.sharding import Mesh, PartitionSpec
        from jax.experimental.shard_map import shard_map
        import concourse.mybir as mybir
        from concourse import bass2jax
        from concourse.bass2jax import _bass_exec_p, partition_id_tensor

        bass2jax.install_neuronx_cc_hook()
        self.jax = jax
        self.nc = nc
        self.n_cores = n_cores
        partition_name = (nc.partition_id_tensor.name
                          if nc.partition_id_tensor else None)
        in_names, out_names, out_avals, zero_outs = [], [], [], []
        for alloc in nc.m.functions[0].allocations:
            if not isinstance(alloc, mybir.MemoryLocationSet):
                continue
            name = alloc.memorylocations[0].name
            if alloc.kind == "ExternalInput":
                if name != partition_name:
                    in_names.append(name)
            elif alloc.kind == "ExternalOutput":
                shape = tuple(alloc.tensor_shape)
                dtype = mybir.dt.np(alloc.dtype)
                out_avals.append(jax.core.ShapedArray(shape, dtype))
                out_names.append(name)
                zero_outs.append(np.zeros(shape, dtype))
        self.in_names = list(in_names)
        self.out_names = out_names
        self.out_avals = out_avals
        self.zero_outs = zero_outs
        n_params = len(self.in_names)
        n_outs = len(out_names)
        all_in_names = self.in_names + out_names
        if partition_name is not None:
            all_in_names.append(partition_name)

        def _body(*args):
            operands = list(args)
            if partition_name is not None:
                operands.append(partition_id_tensor())
            outs = _bass_exec_p.bind(
                *operands,
                out_avals=tuple(out_avals),
                in_names=tuple(all_in_names),
                out_names=tuple(out_names),
                lowering_input_output_aliases=(),
                sim_require_finite=True,
                sim_require_nnan=True,
                nc=nc,
            )
            return tuple(outs)

        devices = jax.devices()[:n_cores]
        self.mesh = Mesh(np.asarray(devices), ("core",))
        in_specs = (PartitionSpec("core"),) * (n_params + n_outs)
        out_specs = (PartitionSpec("core"),) * n_outs
        self.donate = (() if os.environ.get("BASS_NO_DONATE")
                       else tuple(range(n_params, n_params + n_outs)))
        self.fn = jax.jit(
            shard_map(_body, mesh=self.mesh, in_specs=in_specs,
                      out_specs=out_specs, check_rep=False),
            donate_argnums=self.donate, keep_unused=True,
        )
        self.sharding = jax.sharding.NamedSharding(self.mesh,
                                                   PartitionSpec("core"))

    def put_inputs(self, in_maps):
        concat = []
        for name in self.in_names:
            arr = np.concatenate([np.asarray(m[name]) for m in in_maps], axis=0)
            concat.append(self.jax.device_put(arr, self.sharding))
        return concat

    def _zeros(self):
        return [self.jax.device_put(
                    np.zeros((self.n_cores * z.shape[0], *z.shape[1:]), z.dtype),
                    self.sharding)
                for z in self.zero_outs]

    def run(self, dev_inputs):
        outs = self.fn(*dev_inputs, *self._zeros())
        self.jax.block_until_ready(outs)
        return outs

    def time_runs(self, dev_inputs, n_rep=6):
        import time
        ts = []
        for _ in range(n_rep):
            zeros = self._zeros()
            self.jax.block_until_ready(zeros)
            t0 = time.monotonic()
            outs = self.fn(*dev_inputs, *zeros)
            self.jax.block_until_ready(outs)
            ts.append(time.monotonic() - t0)
        return min(ts), ts

    def results(self, outs):
        res = []
        for c in range(self.n_cores):
            d = {}
            for i, name in enumerate(self.out_names):
                d[name] = np.asarray(outs[i]).reshape(
                    self.n_cores, *self.out_avals[i].shape)[c]
            res.append(d)
        return res




_runners = {}
_prep_cache = {}


def _build(meta):
    import concourse.bacc as bacc
    import concourse.tile as tile
    from concourse import mybir
    from concourse.masks import make_identity

    L1, L2 = meta["L1"], meta["L2"]
    nc = bacc.Bacc(None, target_bir_lowering=False, num_devices=NC,
                   num_swdge_queues=4, dynamic_dma_scratch_size=2 ** 15)
    xT = nc.dram_tensor("xT", [F_IN, CHUNK], mybir.dt.float32,
                        kind="ExternalInput")
    w1 = nc.dram_tensor("w1", [F_IN, HID], mybir.dt.float32,
                        kind="ExternalInput")
    w2 = nc.dram_tensor("w2", [HID, N_CLS], mybir.dt.float32,
                        kind="ExternalInput")
    b1d = nc.dram_tensor("b1d", [P, HID], mybir.dt.float32,
                         kind="ExternalInput")
    b2d = nc.dram_tensor("b2d", [P, N_CLS], mybir.dt.float32,
                         kind="ExternalInput")
    dv1d = nc.dram_tensor("dv1d", [P, 4 * NG2], mybir.dt.float32,
                          kind="ExternalInput")
    dv2d = nc.dram_tensor("dv2d", [P, 4 * NG2], mybir.dt.float32,
                          kind="ExternalInput")
    gA1d = nc.dram_tensor("gA1d", [P, L1["TOTCOL"] * 8], mybir.dt.int16,
                          kind="ExternalInput")
    gB1d = nc.dram_tensor("gB1d", [P, L1["COLS2"] * 8], mybir.dt.int16,
                          kind="ExternalInput")
    gA2d = nc.dram_tensor("gA2d", [P, L2["TOTCOL"] * 8], mybir.dt.int16,
                          kind="ExternalInput")
    gB2d = nc.dram_tensor("gB2d", [P, L2["COLS2"] * 8], mybir.dt.int16,
                          kind="ExternalInput")
    outd = nc.dram_tensor("outd", [CHUNK, N_CLS], mybir.dt.float32,
                          kind="ExternalOutput")

    agin1 = [nc.dram_tensor(f"agin1_{k}", [STRIP, F], mybir.dt.float32,
                            kind="Internal") for k in range(4)]
    agin2 = [nc.dram_tensor(f"agin2_{k}", [STRIP, F], mybir.dt.float32,
                            kind="Internal") for k in range(4)]
    table1 = [nc.dram_tensor(f"table1_{k}", [BUCKET_ROWS, F],
                             mybir.dt.float32, kind="Internal",
                             addr_space="Shared") for k in range(4)]
    table2 = [nc.dram_tensor(f"table2_{k}", [BUCKET_ROWS, F],
                             mybir.dt.float32, kind="Internal",
                             addr_space="Shared") for k in range(4)]
    pbuf1 = [nc.dram_tensor(f"pbuf1_{k}", [L1["rows_sd"][k], F],
                            mybir.dt.float32, kind="Internal")
             for k in range(4)]
    pbuf2 = [nc.dram_tensor(f"pbuf2_{k}", [L2["rows_sd"][k], F],
                            mybir.dt.float32, kind="Internal")
             for k in range(4)]

    with tile.TileContext(nc) as tc:
        with tc.tile_pool(name="c0", bufs=1) as cp, \
             tc.tile_pool(name="xs", bufs=2) as xsp, \
             tc.tile_pool(name="mm", bufs=4) as mmp, \
             tc.tile_pool(name="gw", bufs=4) as gwp, \
             tc.tile_pool(name="sb", bufs=4) as sbp, \
             tc.tile_pool(name="gi", bufs=4) as gip, \
             tc.tile_pool(name="g2", bufs=4) as g2p, \
             tc.tile_pool(name="i2", bufs=2) as i2p, \
             tc.tile_pool(name="ep", bufs=4) as epp, \
             tc.tile_pool(name="p1", bufs=4, space="PSUM") as pp1, \
             tc.tile_pool(name="p2", bufs=2, space="PSUM") as pp2:

            # ---- constants
            w1a = cp.tile([P, HID], mybir.dt.float32, tag="w1a")
            w1b = cp.tile([P, HID], mybir.dt.float32, tag="w1b")
            nc.sync.dma_start(out=w1a[:], in_=w1[0:P, :])
            nc.sync.dma_start(out=w1b[:], in_=w1[P:2 * P, :])
            w2t = cp.tile([HID, N_CLS], mybir.dt.float32, tag="w2t")
            nc.sync.dma_start(out=w2t[:], in_=w2[:])
            b1t = cp.tile([P, HID], mybir.dt.float32, tag="b1t")
            nc.sync.dma_start(out=b1t[:], in_=b1d[:])
            b2t = cp.tile([P, N_CLS], mybir.dt.float32, tag="b2t")
            nc.sync.dma_start(out=b2t[:], in_=b2d[:])
            dv1 = cp.tile([P, 4 * NG2], mybir.dt.float32, tag="dv1")
            nc.sync.dma_start(out=dv1[:], in_=dv1d[:])
            dv2 = cp.tile([P, 4 * NG2], mybir.dt.float32, tag="dv2")
            nc.sync.dma_start(out=dv2[:], in_=dv2d[:])
            ident = cp.tile([P, P], mybir.dt.float32, tag="id")
            make_identity(nc, ident[:])

            # ---- layer-1 matmul: hp strips -> agin1 + pbuf1 self block
            sb1 = L1["self_base"]
            for ch in range(10):
                xa = xsp.tile([P, 1280], mybir.dt.float32, tag="xa")
                xb = xsp.tile([P, 1280], mybir.dt.float32, tag="xb")
                nc.sync.dma_start(out=xa[:],
                                  in_=xT[0:P, ch * 1280:(ch + 1) * 1280])
                nc.sync.dma_start(out=xb[:],
                                  in_=xT[P:2 * P, ch * 1280:(ch + 1) * 1280])
                for gl in range(10):
                    g = ch * 10 + gl
                    pst = pp1.tile([P, HID], mybir.dt.float32, tag="mmps")
                    nc.tensor.matmul(out=pst[:],
                                     lhsT=xa[:, gl * P:(gl + 1) * P],
                                     rhs=w1a[:], start=True, stop=False)
                    nc.tensor.matmul(out=pst[:],
                                     lhsT=xb[:, gl * P:(gl + 1) * P],
                                     rhs=w1b[:], start=False, stop=True)
                    ot = mmp.tile([P, HID], mybir.dt.float32, tag="ot")
                    nc.vector.tensor_copy(out=ot[:], in_=pst[:])
                    sd, gl2 = g // 25, g % 25
                    nc.sync.dma_start(
                        out=agin1[sd][gl2 * P:(gl2 + 1) * P, :], in_=ot[:])
                    nc.sync.dma_start(
                        out=pbuf1[sd][sb1[sd] + gl2 * P:
                                      sb1[sd] + (gl2 + 1) * P, :], in_=ot[:])
                    if g % 25 == 24:
                        nc.gpsimd.collective_compute(
                            "AllGather", mybir.AluOpType.bypass,
                            replica_groups=[list(range(NC))],
                            ins=[agin1[sd][:].opt()],
                            outs=[table1[sd][:].opt()])

            # ---- stage A for one (layer, strip): windows
            def stage_a(L, tables, pbufs, sd):
                for w, win in enumerate(L["wins"][sd]):
                    b = win["b"]
                    cols = win["cols"]
                    wc0 = L["woff"][(sd, w)]
                    gsrc = gA1d if L is L1 else gA2d
                    git = gip.tile([P, WCOLS * 8], mybir.dt.int16, tag="git")
                    nc.scalar.dma_start(
                        out=git[:, :cols * 8],
                        in_=gsrc[:, wc0 * 8:(wc0 + cols) * 8])
                    gwt = gwp.tile([P, WCOLS, F], mybir.dt.float32, tag="gw")
                    for cc in range(0, cols, GCALL):
                        k8 = min(GCALL, cols - cc)
                        nc.gpsimd.dma_gather(
                            out_ap=gwt[:, cc:cc + k8, :],
                            in_ap=tables[b][:],
                            idxs_ap=git[:, cc * 8:(cc + k8) * 8],
                            num_idxs=k8 * P, num_idxs_reg=k8 * P,
                            elem_size=F, queue_num=0)
                    nsegs = len(win["segs"])
                    sbt = sbp.tile([P, WCOLS, F], mybir.dt.float32, tag="sb")
                    for r, (g, c0, d) in enumerate(win["segs"]):
                        nc.vector.tensor_reduce(
                            out=sbt[:, r, :],
                            in_=gwt[:, c0:c0 + d, :].rearrange(
                                "p g f -> p f g"),
                            axis=mybir.AxisListType.X,
                            op=mybir.AluOpType.add)
                    g0 = win["segs"][0][0]
                    base = L["gbase"][(b, sd)] + g0 * P
                    nc.sync.dma_start(
                        out=pbufs[sd][base:base + nsegs * P, :].rearrange(
                            "(r p) f -> p r f", p=P),
                        in_=sbt[:, :nsegs, :])

            # ---- stage B layer 1: totals -> relu epilogue -> agin2/pbuf2
            sb2 = L2["self_base"]

            def load_git2(L, gsrc, sd):
                c2 = L["cols2_sd"][sd]
                off = sum(L["cols2_sd"][:sd])
                git2 = i2p.tile([P, max(L["cols2_sd"]) * 8], mybir.dt.int16,
                                tag="git2" + ("a" if L is L1 else "b"))
                nc.scalar.dma_start(out=git2[:, :c2 * 8],
                                    in_=gsrc[:, off * 8:(off + c2) * 8])
                return git2

            WC2 = L1["WC2"]

            def win2_gather(L, pbufs, git2, sd, w, win):
                """Gather one stage-B window; returns the window tile."""
                off = sum(L["cols2_sd"][:sd])
                wc0 = L["woff2"][(sd, w)] - off  # strip-local col base
                cols = win["cols_pad"]
                g2t = g2p.tile([P, WC2, F], mybir.dt.float32, tag="g2t")
                for cc in range(0, cols, GCALL):
                    k8 = min(GCALL, cols - cc)
                    nc.gpsimd.dma_gather(
                        out_ap=g2t[:, cc:cc + k8, :], in_ap=pbufs[sd][:],
                        idxs_ap=git2[:, (wc0 + cc) * 8:(wc0 + cc + k8) * 8],
                        num_idxs=k8 * P, num_idxs_reg=k8 * P,
                        elem_size=F, queue_num=0)
                return g2t

            def seg_total(g2t, c0, D):
                t1 = epp.tile([P, F], mybir.dt.float32, tag="t1")
                nc.vector.tensor_reduce(
                    out=t1[:],
                    in_=g2t[:, c0:c0 + D, :].rearrange("p g f -> p f g"),
                    axis=mybir.AxisListType.X,
                    op=mybir.AluOpType.add)
                return t1

            def stage_b1(sd):
                git2 = load_git2(L1, gB1d, sd)
                for w, win in enumerate(L1["wins2"][sd]):
                    g2t = win2_gather(L1, pbuf1, git2, sd, w, win)
                    for (gl, c0, D) in win["segs"]:
                        g = sd * NG2 + gl
                        dcol = dv1[:, g:g + 1]
                        t1 = seg_total(g2t, c0, D)
                        nc.vector.tensor_tensor(
                            out=t1[:], in0=t1[:],
                            in1=dcol.to_broadcast([P, F]),
                            op=mybir.AluOpType.mult)
                        nc.vector.tensor_tensor(
                            out=t1[:], in0=t1[:], in1=b1t[:],
                            op=mybir.AluOpType.add)
                        t2 = epp.tile([P, F], mybir.dt.float32, tag="t2")
                        nc.scalar.activation(
                            out=t2[:], in_=t1[:],
                            func=mybir.ActivationFunctionType.Relu)
                        nc.vector.tensor_tensor(
                            out=t2[:], in0=t2[:],
                            in1=dcol.to_broadcast([P, F]),
                            op=mybir.AluOpType.mult)
                        nc.sync.dma_start(
                            out=agin2[sd][gl * P:(gl + 1) * P, :], in_=t2[:])
                        nc.sync.dma_start(
                            out=pbuf2[sd][sb2[sd] + gl * P:
                                          sb2[sd] + (gl + 1) * P, :],
                            in_=t2[:])
                nc.gpsimd.collective_compute(
                    "AllGather", mybir.AluOpType.bypass,
                    replica_groups=[list(range(NC))],
                    ins=[agin2[sd][:].opt()], outs=[table2[sd][:].opt()])

            # ---- stage B layer 2: classifier + batched log_softmax
            lgbuf = cp.tile([P, 4 * NG2 * N_CLS], mybir.dt.float32, tag="lgb")
            smT = cp.tile([P, 4 * NG2], mybir.dt.float32, tag="smT")
            lnT = cp.tile([P, 4 * NG2], mybir.dt.float32, tag="lnT")

            def stage_b2(sd):
                git2 = load_git2(L2, gB2d, sd)
                for w, win in enumerate(L2["wins2"][sd]):
                    g2t = win2_gather(L2, pbuf2, git2, sd, w, win)
                    for (gl, c0, D) in win["segs"]:
                        g = sd * NG2 + gl
                        dcol = dv2[:, g:g + 1]
                        t1 = seg_total(g2t, c0, D)
                        pt = pp2.tile([HID, P], mybir.dt.float32, tag="pt")
                        nc.tensor.transpose(out=pt[:], in_=t1[:],
                                            identity=ident[:])
                        zt2 = epp.tile([HID, P], mybir.dt.float32, tag="zt2")
                        nc.vector.tensor_copy(out=zt2[:], in_=pt[:])
                        p2t = pp2.tile([P, N_CLS], mybir.dt.float32, tag="p2")
                        nc.tensor.matmul(out=p2t[:], lhsT=zt2[:], rhs=w2t[:],
                                         start=True, stop=True)
                        lg = lgbuf[:, g * N_CLS:(g + 1) * N_CLS]
                        nc.vector.tensor_tensor(
                            out=lg, in0=p2t[:],
                            in1=dcol.to_broadcast([P, N_CLS]),
                            op=mybir.AluOpType.mult)
                        nc.vector.tensor_tensor(out=lg, in0=lg, in1=b2t[:],
                                                op=mybir.AluOpType.add)
                        mx = epp.tile([P, 1], mybir.dt.float32, tag="mx")
                        nc.vector.tensor_reduce(out=mx[:], in_=lg,
                                                axis=mybir.AxisListType.X,
                                                op=mybir.AluOpType.max)
                        nc.vector.tensor_tensor(
                            out=lg, in0=lg,
                            in1=mx[:].to_broadcast([P, N_CLS]),
                            op=mybir.AluOpType.subtract)
                        ex = epp.tile([P, N_CLS], mybir.dt.float32, tag="ex")
                        nc.scalar.activation(
                            out=ex[:], in_=lg,
                            func=mybir.ActivationFunctionType.Exp)
                        nc.vector.tensor_reduce(out=smT[:, g:g + 1],
                                                in_=ex[:],
                                                axis=mybir.AxisListType.X,
                                                op=mybir.AluOpType.add)
                nc.scalar.activation(
                    out=lnT[:, sd * NG2:(sd + 1) * NG2],
                    in_=smT[:, sd * NG2:(sd + 1) * NG2],
                    func=mybir.ActivationFunctionType.Ln)
                for gl in range(NG2):
                    g = sd * NG2 + gl
                    og = epp.tile([P, N_CLS], mybir.dt.float32, tag="og")
                    nc.vector.tensor_tensor(
                        out=og[:], in0=lgbuf[:, g * N_CLS:(g + 1) * N_CLS],
                        in1=lnT[:, g:g + 1].to_broadcast([P, N_CLS]),
                        op=mybir.AluOpType.subtract)
                    nc.sync.dma_start(
                        out=outd[(sd * STRIP + gl * P):
                                 (sd * STRIP + (gl + 1) * P), :],
                        in_=og[:])

            # ---- emission: stage B pipelined one strip behind stage A
            stage_a(L1, table1, pbuf1, 0)
            stage_a(L1, table1, pbuf1, 1)
            stage_b1(0)
            stage_a(L1, table1, pbuf1, 2)
            stage_b1(1)
            stage_a(L1, table1, pbuf1, 3)
            stage_b1(2)
            stage_b1(3)
            stage_a(L2, table2, pbuf2, 0)
            stage_a(L2, table2, pbuf2, 1)
            stage_b2(0)
            stage_a(L2, table2, pbuf2, 2)
            stage_b2(1)
            stage_a(L2, table2, pbuf2, 3)
            stage_b2(2)
            stage_b2(3)

    # Route each gather to the SWDGE queue matching its Tile-assigned DMASW
    # completion lane (lane k -> queue k%4).  Tile assigns lanes round-robin
    # over the *scheduled* Pool-DMA order, which the scheduler may permute
    # relative to emission order; per-queue completion is in-order, so a lane
    # must only ever be fed from one queue.
    import re as _re
    for ins in nc.all_instructions():
        if type(ins).__name__ != "InstDMAGatherAnt":
            continue
        si = getattr(ins, "sync_info", None)
        ups = list(getattr(si, "on_update", None) or []) if si else []
        lane = None
        for u in ups:
            m = _re.match(r"DMASW(\d+)_", getattr(u, "ant_name", "") or "")
            if m:
                lane = int(m.group(1))
                break
        assert lane is not None, f"no DMASW lane on {ins.name}"
        ins.queue_num = lane % 4
    nc.finalize()
    return nc


def _prep(edge_index):
    key = hashlib.sha1(np.ascontiguousarray(edge_index).tobytes()).hexdigest()
    if key not in _prep_cache:
        _prep_cache[key] = build_schedule(edge_index)
    return _prep_cache[key]


def _meta_key(meta):
    h = hashlib.sha1()
    for L in (meta["L1"], meta["L2"]):
        h.update(repr([L["TOTCOL"], L["COLS2"], L["cols2_sd"],
                       L["rows_sd"], sorted(L["gbase"].items()),
                       L["D2s"].tolist(),
                       [(sd, w["b"], w["cols"], tuple(map(tuple, w["segs"])))
                        for sd in range(4) for w in L["wins"][sd]],
                       [(sd, w["cols"], w["cols_pad"],
                         tuple(map(tuple, w["segs"])))
                        for sd in range(4) for w in L["wins2"][sd]]]).encode())
    return h.hexdigest()


def get_runner(meta):
    key = _meta_key(meta)
    if key not in _runners:
        _runners[key] = BassRunner(_build(meta), NC)
    return _runners[key]


def make_in_maps(x, W1, b1, W2, b2, meta):
    x = np.asarray(x, np.float32)
    dinv = meta["dinv"]
    L1, L2 = meta["L1"], meta["L2"]
    b1r = np.tile(np.asarray(b1, np.float32)[None, :], (P, 1))
    b2r = np.tile(np.asarray(b2, np.float32)[None, :], (P, 1))
    xs_all = (x * dinv[:, None]).astype(np.float32)
    in_maps = []
    for c in range(NC):
        xs = np.zeros((F_IN, CHUNK), np.float32)
        for sd in range(4):
            n0 = c * PER_CORE + sd * SJ
            xs[:, sd * STRIP:sd * STRIP + SJ] = xs_all[n0:n0 + SJ].T
        in_maps.append({
            "xT": xs, "w1": np.asarray(W1, np.float32),
            "w2": np.asarray(W2, np.float32), "b1d": b1r, "b2d": b2r,
            "dv1d": L1["dv_out"][c], "dv2d": L2["dv_out"][c],
            "gA1d": L1["gidx"][c], "gB1d": L1["gidx2"][c],
            "gA2d": L2["gidx"][c], "gB2d": L2["gidx2"][c]})
    return in_maps


def assemble_out(res, meta):
    outrow = meta["outrow"]
    out = np.empty((N_NODES, N_CLS), np.float32)
    for c in range(NC):
        out[c * PER_CORE:(c + 1) * PER_CORE] = res[c]["outd"][outrow[c]]
    return out


def kernel(x, edge_index, pos_edge_index, neg_edge_index, masked_nodes,
           W1, b1, W2, b2):
    meta = _prep(np.asarray(edge_index))
    runner = get_runner(meta)
    in_maps = make_in_maps(x, W1, b1, W2, b2, meta)
    res = runner.results(runner.run(runner.put_inputs(in_maps)))
    return assemble_out(res, meta)
